# revision 7
# baseline (speedup 1.0000x reference)
"""Causal multi-head attention (S=2048, B=2, H=16, D=128, fp32) on 8 trn2 cores.

Sharding: the 32 (batch, head) pairs are split 4-per-core (tensor parallel on
heads). Each core runs a flash-attention-style kernel in the "S^T layout",
processing key blocks two at a time:

  For a query chunk c (512 wide) and key-block pair (j0, j1) (128 wide each):
    S^T[k, q] = matmul: lhsT = K^T[d, k_j], rhs = Q^T[d, q_c]   (PE, fp16) x2
    P^T = exp(S^T)            (Q pre-scaled by 1/sqrt(D) on host)  (ACT, fp16)
    causal mask via affine_select (keep where q >= k, else 0)   (GpSimd)
    ctx^T[d, q_c] += matmul: lhsT = V[k_j, d], rhs = P^T        (PE, fp16) x2
    l[q_c]: even-position pairs accumulate into PSUM via a PE ones-matmul;
          odd-position pairs accumulate P on the DVE (pacc), folded into
          l by one matmul per chunk.  (fp8 P/V was tried: 1.6-3.5%% rel
          err from e4m3 weight rounding busts the 2%% gate.)

  Diagonal pairs are processed FIRST within a chunk so the GpSimd mask
  latency hides behind the off-diagonal QK/exp stream. After the mask the
  whole P tile is clean (zeros in masked/stale cols), so PV/l run full
  width and the PSUM accumulation group stays trivially well-formed.

Host pre-transposes Q/K to [d, s] per head (fp16) and V to the
[k_part, j, d] weight layout (fp16), and does the final divide ctx/l.
"""

import sys

if "/opt/trn_rl_repo" not in sys.path:
    sys.path.insert(0, "/opt/trn_rl_repo")

import numpy as np

S, B, H, D = 2048, 2, 16, 128
N_CORES = 8
HPC = (B * H) // N_CORES  # head-slices per core = 4
QCH = 512  # query chunk width (one PSUM bank of fp32 per k-block)
NCH = S // QCH  # 4 chunks
NKB = S // 128  # 16 key blocks
SCALE = 1.0 / float(np.sqrt(D))

_compiled = None


def _build():
    import concourse.tile as tile
    from concourse import bacc, mybir

    f32 = mybir.dt.float32
    f16 = mybir.dt.float16

    nc = bacc.Bacc("TRN2", target_bir_lowering=False, debug=False)
    qT = nc.dram_tensor("qT", [HPC, D, S], f16, kind="ExternalInput").ap()
    kT = nc.dram_tensor("kT", [HPC, D, S], f16, kind="ExternalInput").ap()
    # v pre-swizzled on host: [head, k_partition (128), j (NKB), d (128)]
    v = nc.dram_tensor("v", [HPC, 128, NKB, D], f16, kind="ExternalInput").ap()
    out = nc.dram_tensor("out", [HPC, D, S], f16, kind="ExternalOutput").ap()
    lsum = nc.dram_tensor("lsum", [HPC, S], f32, kind="ExternalOutput").ap()

    with tile.TileContext(nc) as tc:
        with (
            tc.tile_pool(name="const", bufs=1) as const_pool,
            tc.tile_pool(name="io", bufs=2) as io_pool,
            tc.tile_pool(name="p", bufs=5) as p_pool,
            tc.tile_pool(name="acc", bufs=2) as acc_pool,
            tc.tile_pool(name="o", bufs=3) as o_pool,
            tc.tile_pool(name="psum_s", bufs=3, space="PSUM") as psum_s,
            tc.tile_pool(name="psum_ctx", bufs=1, space="PSUM") as psum_ctx,
            tc.tile_pool(name="psum_l", bufs=1, space="PSUM") as psum_l,
        ):
            ones_f32 = const_pool.tile([128, 1], f32)
            nc.vector.memset(ones_f32[:], 1.0)
            ones16 = const_pool.tile([128, 1], f16)
            nc.vector.tensor_copy(ones16[:], ones_f32[:])

            for h in range(HPC):
                # chunked loads so chunk-0 compute starts before the whole
                # head is resident
                qT_s = io_pool.tile([128, S], f16, tag="qT_s")
                kT_s = io_pool.tile([128, S], f16, tag="kT_s")
                v_s = io_pool.tile([128, NKB, 128], f16, tag="v_s")
                for c in range(NCH):
                    sl = slice(c * QCH, (c + 1) * QCH)
                    if h == 0 and c == 0:
                        # first compute needs kT[0:256] + all of qT chunk 0;
                        # order the pieces so it can start ASAP
                        nc.sync.dma_start(kT_s[:, 0:256], kT[h][:, 0:256])
                        nc.sync.dma_start(qT_s[:, sl], qT[h][:, sl])
                        nc.sync.dma_start(kT_s[:, 256:512], kT[h][:, 256:512])
                    else:
                        nc.sync.dma_start(kT_s[:, sl], kT[h][:, sl])
                        nc.sync.dma_start(qT_s[:, sl], qT[h][:, sl])
                    nc.sync.dma_start(
                        v_s[:, 4 * c : 4 * c + 4, :], v[h][:, 4 * c : 4 * c + 4, :]
                    )

                chunk_order = range(NCH) if h == 0 else range(NCH - 1, -1, -1)
                for c in chunk_order:
                    qmov = qT_s[:, c * QCH : (c + 1) * QCH]
                    ctx_c = psum_ctx.tile([128, QCH], f32, tag="ctx")
                    l_c = psum_l.tile([1, QCH], f32, tag="l")
                    pacc = acc_pool.tile([128, QCH], f16, tag="pacc")
                    npairs = 2 * c + 2
                    # diagonal pairs (2c, 2c+1) first: their GpSimd mask
                    # latency hides behind the off-diagonal stream
                    order = [2 * c, 2 * c + 1] + list(range(0, 2 * c))
                    for idx, pi in enumerate(order):
                        j0, j1 = 2 * pi, 2 * pi + 1
                        # causal trim: for block j, q columns < 128(j-4c) are
                        # fully masked; skip them at 128-col granularity
                        # (fp16 PE runs full rate at any moving width)
                        w = [
                            min(max(0, 128 * (j - 4 * c)), QCH - 128)
                            for j in (j0, j1)
                        ]
                        diag = j1 >= 4 * c
                        s2 = psum_s.tile([128, 2, QCH], f32, tag="s2")
                        p2 = p_pool.tile([128, 2, QCH], f16, tag="p2")
                        for o, j in enumerate((j0, j1)):
                            nc.tensor.matmul(
                                s2[:, o, w[o] :],
                                kT_s[:, j * 128 : (j + 1) * 128],
                                qmov[:, w[o] :],
                                start=True,
                                stop=True,
                            )
                        nc.scalar.activation(
                            p2[:, :, w[0] :],
                            s2[:, :, w[0] :],
                            mybir.ActivationFunctionType.Exp,
                        )
                        if diag:
                            # keep where q_global >= k_global; for col x of
                            # half o (j = j0+o): iota = (512c + x)
                            # - 128(j0+o) - part.  Also fills the skipped
                            # (stale) prefix columns with 0, so the whole
                            # tile is clean afterwards.
                            nc.gpsimd.affine_select(
                                p2[:],
                                p2[:],
                                pattern=[[-128, 2], [1, QCH]],
                                base=c * QCH - j0 * 128,
                                channel_multiplier=-1,
                                compare_op=mybir.AluOpType.is_ge,
                                fill=0.0,
                            )
                        for o, j in enumerate((j0, j1)):
                            nc.tensor.matmul(
                                ctx_c[:, w[o] :],
                                v_s[:, j, :],
                                p2[:, o, w[o] :],
                                start=(idx == 0 and o == 0),
                                stop=(idx == npairs - 1 and o == 1),
                                skip_group_check=True,
                            )
                        if idx % 2 == 0:
                            # even slots: l via PE ones-matmul into PSUM
                            for o in range(2):
                                nc.tensor.matmul(
                                    l_c[:, w[o] :],
                                    ones16[:],
                                    p2[:, o, w[o] :],
                                    start=(idx == 0 and o == 0),
                                    stop=False,
                                    skip_group_check=True,
                                )
                        else:
                            # odd slots: accumulate P on DVE; folded into l_c
                            # by one matmul per chunk at the end
                            if idx == 1:
                                nc.vector.tensor_add(
                                    pacc[:], p2[:, 0, :], p2[:, 1, :]
                                )
                            else:
                                nc.vector.tensor_add(
                                    pacc[:], pacc[:], p2[:, 0, :]
                                )
                                nc.vector.tensor_add(
                                    pacc[:], pacc[:], p2[:, 1, :]
                                )
                    nc.tensor.matmul(
                        l_c[:],
                        ones16[:],
                        pacc[:],
                        start=False,
                        stop=True,
                        skip_group_check=True,
                    )
                    o_t = o_pool.tile([128, QCH], f16, tag="o")
                    nc.vector.tensor_copy(o_t[:], ctx_c[:])
                    nc.sync.dma_start(
                        out[h][:, c * QCH : (c + 1) * QCH], o_t[:]
                    )
                    lo_t = o_pool.tile([1, QCH], f32, tag="lo")
                    nc.vector.tensor_copy(lo_t[:], l_c[0:1])
                    nc.sync.dma_start(
                        lsum[h : h + 1, c * QCH : (c + 1) * QCH], lo_t[:]
                    )

    nc.compile()
    return nc


def _get_compiled():
    global _compiled
    if _compiled is None:
        _compiled = _build()
    return _compiled


def _run(query_layer, key_layer, value_layer, attention_mask=None, trace=False):
    from concourse import bass_utils

    nc = _get_compiled()

    q = np.asarray(query_layer, dtype=np.float32)
    k = np.asarray(key_layer, dtype=np.float32)
    v = np.asarray(value_layer, dtype=np.float32)

    # [S,B,H,D] -> [BH, D, S] for q/k (fp16).
    # Fold the 1/sqrt(D) softmax scale into Q on the host.
    qT_all = np.ascontiguousarray(
        (q.transpose(1, 2, 3, 0).reshape(B * H, D, S) * np.float32(SCALE)).astype(
            np.float16
        )
    )
    kT_all = np.ascontiguousarray(
        k.transpose(1, 2, 3, 0).reshape(B * H, D, S).astype(np.float16)
    )
    # v: [S,B,H,D] -> [BH, k_part(128), j(NKB), D] fp8 (DoubleRow weights
    # layout: s = j*128 + k_part)
    v_all = np.ascontiguousarray(
        v.transpose(1, 2, 0, 3)
        .reshape(B * H, NKB, 128, D)
        .transpose(0, 2, 1, 3)
        .astype(np.float16)
    )

    in_maps = [
        {
            "qT": qT_all[c * HPC : (c + 1) * HPC],
            "kT": kT_all[c * HPC : (c + 1) * HPC],
            "v": v_all[c * HPC : (c + 1) * HPC],
        }
        for c in range(N_CORES)
    ]
    res = bass_utils.run_bass_kernel_spmd(
        nc, in_maps, list(range(N_CORES)), trace=trace
    )

    ctxT = np.concatenate(
        [np.asarray(res.results[c]["out"], dtype=np.float32) for c in range(N_CORES)],
        axis=0,
    )  # [BH, D, S]
    l = np.concatenate(
        [res.results[c]["lsum"] for c in range(N_CORES)], axis=0
    )  # [BH, S]
    ctxT = ctxT / l[:, None, :]
    # [BH, D, S] -> [S, B, H*D]
    full = ctxT.reshape(B, H, D, S).transpose(3, 0, 1, 2).reshape(S, B, H * D)
    return np.ascontiguousarray(full.astype(np.float32)), res


def kernel(query_layer, key_layer, value_layer, attention_mask=None):
    out, _ = _run(query_layer, key_layer, value_layer, attention_mask)
    return out


# revision 8
# speedup vs baseline: 1.0521x; 1.0521x over previous
"""Causal multi-head attention (S=2048, B=2, H=16, D=128, fp32) on 8 trn2 cores.

Sharding: the 32 (batch, head) pairs are split 4-per-core (tensor parallel on
heads). Each core runs a flash-attention-style kernel in the "S^T layout",
processing key blocks two at a time:

  For a query chunk c (512 wide) and key-block pair (j0, j1) (128 wide each):
    S^T[k, q] = matmul: lhsT = K^T[d, k_j], rhs = Q^T[d, q_c]   (PE, fp16) x2
    P^T = exp(S^T)            (Q pre-scaled by 1/sqrt(D) on host)  (ACT, fp16)
    causal mask via affine_select (keep where q >= k, else 0)   (GpSimd)
    ctx^T[d, q_c] += matmul: lhsT = V[k_j, d], rhs = P^T        (PE, fp16) x2
    l[q_c]: even-position pairs accumulate into PSUM via a PE ones-matmul;
          odd-position pairs accumulate P on the DVE (pacc), folded into
          l by one matmul per chunk.  (fp8 P/V was tried: 1.6-3.5%% rel
          err from e4m3 weight rounding busts the 2%% gate.)

  Diagonal pairs are processed FIRST within a chunk so the GpSimd mask
  latency hides behind the off-diagonal QK/exp stream. After the mask the
  whole P tile is clean (zeros in masked/stale cols), so PV/l run full
  width and the PSUM accumulation group stays trivially well-formed.

Host pre-transposes Q/K to [d, s] per head (fp16) and V to the
[k_part, j, d] weight layout (fp16), and does the final divide ctx/l.
"""

import sys

if "/opt/trn_rl_repo" not in sys.path:
    sys.path.insert(0, "/opt/trn_rl_repo")

import numpy as np

S, B, H, D = 2048, 2, 16, 128
N_CORES = 8
HPC = (B * H) // N_CORES  # head-slices per core = 4
QCH = 512  # query chunk width (one PSUM bank of fp32 per k-block)
NCH = S // QCH  # 4 chunks
NKB = S // 128  # 16 key blocks
SCALE = 1.0 / float(np.sqrt(D))

_compiled = None


def _build():
    import concourse.tile as tile
    from concourse import bacc, mybir

    f32 = mybir.dt.float32
    f16 = mybir.dt.float16

    nc = bacc.Bacc("TRN2", target_bir_lowering=False, debug=False)
    qT = nc.dram_tensor("qT", [HPC, D, S], f16, kind="ExternalInput").ap()
    kT = nc.dram_tensor("kT", [HPC, D, S], f16, kind="ExternalInput").ap()
    # v pre-swizzled on host: [head, k_partition (128), j (NKB), d (128)]
    v = nc.dram_tensor("v", [HPC, 128, NKB, D], f16, kind="ExternalInput").ap()
    out = nc.dram_tensor("out", [HPC, D, S], f16, kind="ExternalOutput").ap()
    lsum = nc.dram_tensor("lsum", [HPC, S], f32, kind="ExternalOutput").ap()

    with tile.TileContext(nc) as tc:
        with (
            tc.tile_pool(name="const", bufs=1) as const_pool,
            tc.tile_pool(name="io", bufs=2) as io_pool,
            tc.tile_pool(name="p", bufs=5) as p_pool,
            tc.tile_pool(name="acc", bufs=2) as acc_pool,
            tc.tile_pool(name="o", bufs=3) as o_pool,
            tc.tile_pool(name="psum_s", bufs=3, space="PSUM") as psum_s,
            tc.tile_pool(name="psum_ctx", bufs=1, space="PSUM") as psum_ctx,
            tc.tile_pool(name="psum_l", bufs=1, space="PSUM") as psum_l,
        ):
            ones_f32 = const_pool.tile([128, 1], f32)
            nc.vector.memset(ones_f32[:], 1.0)
            ones16 = const_pool.tile([128, 1], f16)
            nc.vector.tensor_copy(ones16[:], ones_f32[:])
            # preload the Exp activation table while the first DMAs fly
            warm = const_pool.tile([128, 1], f16)
            nc.scalar.activation(
                warm[:], ones_f32[:], mybir.ActivationFunctionType.Exp
            )

            for h in range(HPC):
                # chunked loads so chunk-0 compute starts before the whole
                # head is resident
                qT_s = io_pool.tile([128, S], f16, tag="qT_s")
                kT_s = io_pool.tile([128, S], f16, tag="kT_s")
                v_s = io_pool.tile([128, NKB, 128], f16, tag="v_s")
                for c in range(NCH):
                    sl = slice(c * QCH, (c + 1) * QCH)
                    if h == 0 and c == 0:
                        # first compute needs kT[0:256] + all of qT chunk 0;
                        # order the pieces so it can start ASAP
                        nc.sync.dma_start(kT_s[:, 0:128], kT[h][:, 0:128])
                        nc.sync.dma_start(qT_s[:, sl], qT[h][:, sl])
                        nc.sync.dma_start(kT_s[:, 128:256], kT[h][:, 128:256])
                        nc.sync.dma_start(kT_s[:, 256:512], kT[h][:, 256:512])
                    else:
                        nc.sync.dma_start(kT_s[:, sl], kT[h][:, sl])
                        nc.sync.dma_start(qT_s[:, sl], qT[h][:, sl])
                    nc.sync.dma_start(
                        v_s[:, 4 * c : 4 * c + 4, :], v[h][:, 4 * c : 4 * c + 4, :]
                    )

                chunk_order = range(NCH) if h == 0 else range(NCH - 1, -1, -1)
                for c in chunk_order:
                    qmov = qT_s[:, c * QCH : (c + 1) * QCH]
                    ctx_c = psum_ctx.tile([128, QCH], f32, tag="ctx")
                    l_c = psum_l.tile([1, QCH], f32, tag="l")
                    pacc = acc_pool.tile([128, QCH], f16, tag="pacc")
                    npairs = 2 * c + 2
                    # diagonal pairs (2c, 2c+1) first: their GpSimd mask
                    # latency hides behind the off-diagonal stream
                    order = [2 * c, 2 * c + 1] + list(range(0, 2 * c))
                    for idx, pi in enumerate(order):
                        j0, j1 = 2 * pi, 2 * pi + 1
                        # causal trim: for block j, q columns < 128(j-4c) are
                        # fully masked; skip them at 128-col granularity
                        # (fp16 PE runs full rate at any moving width)
                        w = [
                            min(max(0, 128 * (j - 4 * c)), QCH - 128)
                            for j in (j0, j1)
                        ]
                        diag = j1 >= 4 * c
                        s2 = psum_s.tile([128, 2, QCH], f32, tag="s2")
                        p2 = p_pool.tile([128, 2, QCH], f16, tag="p2")
                        for o, j in enumerate((j0, j1)):
                            nc.tensor.matmul(
                                s2[:, o, w[o] :],
                                kT_s[:, j * 128 : (j + 1) * 128],
                                qmov[:, w[o] :],
                                start=True,
                                stop=True,
                            )
                        nc.scalar.activation(
                            p2[:, :, w[0] :],
                            s2[:, :, w[0] :],
                            mybir.ActivationFunctionType.Exp,
                        )
                        if diag:
                            # keep where q_global >= k_global; for col x of
                            # half o (j = j0+o): iota = (512c + x)
                            # - 128(j0+o) - part.  Also fills the skipped
                            # (stale) prefix columns with 0, so the whole
                            # tile is clean afterwards.
                            nc.gpsimd.affine_select(
                                p2[:],
                                p2[:],
                                pattern=[[-128, 2], [1, QCH]],
                                base=c * QCH - j0 * 128,
                                channel_multiplier=-1,
                                compare_op=mybir.AluOpType.is_ge,
                                fill=0.0,
                            )
                        for o, j in enumerate((j0, j1)):
                            nc.tensor.matmul(
                                ctx_c[:, w[o] :],
                                v_s[:, j, :],
                                p2[:, o, w[o] :],
                                start=(idx == 0 and o == 0),
                                stop=(idx == npairs - 1 and o == 1),
                                skip_group_check=True,
                            )
                        if idx % 3 == 0:
                            # every 3rd slot: l via PE ones-matmul into PSUM
                            for o in range(2):
                                nc.tensor.matmul(
                                    l_c[:, w[o] :],
                                    ones16[:],
                                    p2[:, o, w[o] :],
                                    start=(idx == 0 and o == 0),
                                    stop=False,
                                    skip_group_check=True,
                                )
                        else:
                            # other slots: accumulate P on DVE; folded into
                            # l_c by one matmul per chunk at the end
                            if idx == 1:
                                nc.vector.tensor_add(
                                    pacc[:], p2[:, 0, :], p2[:, 1, :]
                                )
                            else:
                                nc.vector.tensor_add(
                                    pacc[:], pacc[:], p2[:, 0, :]
                                )
                                nc.vector.tensor_add(
                                    pacc[:], pacc[:], p2[:, 1, :]
                                )
                    nc.tensor.matmul(
                        l_c[:],
                        ones16[:],
                        pacc[:],
                        start=False,
                        stop=True,
                        skip_group_check=True,
                    )
                    o_t = o_pool.tile([128, QCH], f16, tag="o")
                    nc.vector.tensor_copy(o_t[:], ctx_c[:])
                    nc.sync.dma_start(
                        out[h][:, c * QCH : (c + 1) * QCH], o_t[:]
                    )
                    lo_t = o_pool.tile([1, QCH], f32, tag="lo")
                    nc.vector.tensor_copy(lo_t[:], l_c[0:1])
                    nc.sync.dma_start(
                        lsum[h : h + 1, c * QCH : (c + 1) * QCH], lo_t[:]
                    )

    nc.compile()
    return nc


def _get_compiled():
    global _compiled
    if _compiled is None:
        _compiled = _build()
    return _compiled


def _run(query_layer, key_layer, value_layer, attention_mask=None, trace=False):
    from concourse import bass_utils

    nc = _get_compiled()

    q = np.asarray(query_layer, dtype=np.float32)
    k = np.asarray(key_layer, dtype=np.float32)
    v = np.asarray(value_layer, dtype=np.float32)

    # [S,B,H,D] -> [BH, D, S] for q/k (fp16).
    # Fold the 1/sqrt(D) softmax scale into Q on the host.
    qT_all = np.ascontiguousarray(
        (q.transpose(1, 2, 3, 0).reshape(B * H, D, S) * np.float32(SCALE)).astype(
            np.float16
        )
    )
    kT_all = np.ascontiguousarray(
        k.transpose(1, 2, 3, 0).reshape(B * H, D, S).astype(np.float16)
    )
    # v: [S,B,H,D] -> [BH, k_part(128), j(NKB), D] fp8 (DoubleRow weights
    # layout: s = j*128 + k_part)
    v_all = np.ascontiguousarray(
        v.transpose(1, 2, 0, 3)
        .reshape(B * H, NKB, 128, D)
        .transpose(0, 2, 1, 3)
        .astype(np.float16)
    )

    in_maps = [
        {
            "qT": qT_all[c * HPC : (c + 1) * HPC],
            "kT": kT_all[c * HPC : (c + 1) * HPC],
            "v": v_all[c * HPC : (c + 1) * HPC],
        }
        for c in range(N_CORES)
    ]
    res = bass_utils.run_bass_kernel_spmd(
        nc, in_maps, list(range(N_CORES)), trace=trace
    )

    ctxT = np.concatenate(
        [np.asarray(res.results[c]["out"], dtype=np.float32) for c in range(N_CORES)],
        axis=0,
    )  # [BH, D, S]
    l = np.concatenate(
        [res.results[c]["lsum"] for c in range(N_CORES)], axis=0
    )  # [BH, S]
    ctxT = ctxT / l[:, None, :]
    # [BH, D, S] -> [S, B, H*D]
    full = ctxT.reshape(B, H, D, S).transpose(3, 0, 1, 2).reshape(S, B, H * D)
    return np.ascontiguousarray(full.astype(np.float32)), res


def kernel(query_layer, key_layer, value_layer, attention_mask=None):
    out, _ = _run(query_layer, key_layer, value_layer, attention_mask)
    return out
